# revision 4
# baseline (speedup 1.0000x reference)
"""Trainium2 Bass kernel for nn_DAGAM_24206435680718.

Computes, per batch b of x [B=16, N=8192, C=128]:
  x1, x2 = halves of x along tokens
  q,k1,v1 = qkv(x1); k2,v2 = qkv(x2)   (per-token, heads h=8, hd=16; C split as (hd, h))
  per-token 8x8 head attention: attn1 = softmax(q k1^T) v1 ; attn2 = softmax(q k2^T) v2
  SE channel gates from mean/max over tokens of attn_cm/attn1_cm/attn2_cm
  A = softmax_c(ch ch1^T), AA = softmax_c(ch ch2^T)
  out = concat(A^T attn_cm + attn1_cm, AA^T attn_cm + attn2_cm) -> proj

Sharding: data-parallel over batch. 8 cores x 2 batches each. Params replicated.

Layout strategy per 128-token tile:
  - PE transposes x tiles to channel-major, PE does qkv as one matmul per tile
    (weights stationary in SBUF, columns pre-permuted so q/k come out head-major
    (g,d) and v stays head-minor (d,g)).
  - The per-token heads-attention runs on DVE in token-major layout using
    broadcast access patterns: products (g,g',d) -> reduce over d -> exp (ACT)
    -> normalize -> products (g,d,g') -> reduce over g'.
  - attn tiles are PE-transposed back to channel-major; channel sums ride the
    ACT PSUM->SBUF copies via accum_out; maxes via running DVE max.
  - Final projection fused: out1 = attn_cm^T (A proj_w) + attn1_cm^T proj_w + b,
    three PSUM-accumulated matmuls per output tile (bias via K=1 outer product).
"""

import numpy as np

B, N, C = 16, 8192, 128
H, HD = 8, 16
M = N // 2            # 4096 tokens per half
TT = 128              # tokens per tile
NT = M // TT          # 32 tiles per half
NCORES = 8
BLOC = B // NCORES    # batches per core

_CACHE = {}


def _build():
    import concourse.bass as bass
    import concourse.tile as tile
    from concourse import mybir, bacc
    from concourse.masks import make_identity

    f32 = mybir.dt.float32
    bf16 = mybir.dt.bfloat16
    AF = mybir.ActivationFunctionType
    ALU = mybir.AluOpType
    AX = mybir.AxisListType

    nc = bacc.Bacc("TRN2", target_bir_lowering=False, debug=False)

    x_d = nc.dram_tensor("x", [BLOC, N, C], f32, kind="ExternalInput")
    qkvw_d = nc.dram_tensor("qkv_w", [C, 3 * C], f32, kind="ExternalInput")
    caw1_d = nc.dram_tensor("ca_w1", [C // 4, C], f32, kind="ExternalInput")
    caw2_d = nc.dram_tensor("ca_w2", [C, C // 4], f32, kind="ExternalInput")
    projw_d = nc.dram_tensor("proj_w", [C, C], f32, kind="ExternalInput")
    projb_d = nc.dram_tensor("proj_b", [C], f32, kind="ExternalInput")
    y_d = nc.dram_tensor("y", [BLOC, N, C], f32, kind="ExternalOutput")

    with tile.TileContext(nc) as tc:
        with (
            tc.tile_pool(name="persist", bufs=1) as pp,
            tc.tile_pool(name="xin", bufs=3) as xin,
            tc.tile_pool(name="work", bufs=3) as wk,
            tc.tile_pool(name="att", bufs=2) as at,
            tc.tile_pool(name="outp", bufs=3) as op_pool,
            tc.tile_pool(name="psum", bufs=2, space="PSUM") as ps,
            tc.tile_pool(name="psum1", bufs=1, space="PSUM") as ps1,
        ):
            # ---------------- setup: weights ----------------
            identf = pp.tile([128, 128], f32)
            make_identity(nc, identf[:])

            w32 = pp.tile([C, 3 * C], f32)
            nc.sync.dma_start(w32[:], qkvw_d[:])
            wqkv = pp.tile([C, 3 * C], bf16)  # cols: q head-major | k head-major | v orig
            # q block: dest (g,d) <- src col d*8+g
            nc.vector.tensor_copy(
                wqkv[:, 0:C].rearrange("p (g d) -> p g d", g=H),
                w32[:, 0:C].rearrange("p (d g) -> p d g", d=HD).transpose([0, 2, 1]),
            )
            nc.vector.tensor_copy(
                wqkv[:, C:2 * C].rearrange("p (g d) -> p g d", g=H),
                w32[:, C:2 * C].rearrange("p (d g) -> p d g", d=HD).transpose([0, 2, 1]),
            )
            nc.vector.tensor_copy(wqkv[:, 2 * C:3 * C], w32[:, 2 * C:3 * C])

            projp32 = pp.tile([C, C], f32)
            nc.sync.dma_start(projp32[:], projw_d[:])
            projp = pp.tile([C, C], bf16)
            nc.vector.tensor_copy(projp[:], projp32[:])

            # ca_w1^T [c, 32] and ca_w2^T [32, c] via PE transpose (original channel order)
            caw1s = pp.tile([C // 4, C], f32)
            nc.sync.dma_start(caw1s[:], caw1_d[:])
            pst1 = ps.tile([C, C // 4], f32, tag="kv2")
            nc.tensor.transpose(pst1[:], caw1s[:], identf[0:C // 4, 0:C // 4])
            caw1t = pp.tile([C, C // 4], bf16)
            nc.vector.tensor_copy(caw1t[:], pst1[:])

            caw2s = pp.tile([C, C // 4], f32)
            nc.sync.dma_start(caw2s[:], caw2_d[:])
            pst2 = ps.tile([C // 4, C], f32, tag="kv2")
            nc.tensor.transpose(pst2[:], caw2s[:], identf[:])
            caw2t = pp.tile([C // 4, C], bf16)
            nc.vector.tensor_copy(caw2t[:], pst2[:])

            projb_row32 = pp.tile([1, C], f32)
            nc.sync.dma_start(projb_row32[:], projb_d[:].unsqueeze(0))
            projb_row = pp.tile([1, C], bf16)
            nc.vector.tensor_copy(projb_row[:], projb_row32[:])
            ones_row = pp.tile([1, C], bf16)
            nc.gpsimd.memset(ones_row[:], 1.0)

            # persistent per-batch buffers
            a1cm = pp.tile([C, M], bf16)
            a2cm = pp.tile([C, M], bf16)
            acm = pp.tile([C, M], bf16)
            sum1c = pp.tile([C, NT], f32)
            sum2c = pp.tile([C, NT], f32)
            mx1a = pp.tile([C, TT], bf16)
            mx2a = pp.tile([C, TT], bf16)
            mxca = pp.tile([C, TT], bf16)

            def attention(qkv_sb, koff, voff, attn_out):
                """Per-token 8-head attention for one tile (token-major).

                qkv_sb: [128, *] bf16 with q at cols 0:128 of the x1 qkv tile,
                k at koff, v at voff (within their source tile).
                attn_out: [128, 128] f32 out (g,d) head-major."""
                q_ap, k_sb = qkv_sb
                prod = wk.tile([TT, H * H * HD], bf16, tag="prod")
                s = wk.tile([TT, H * H], f32, tag="s")
                e = wk.tile([TT, H * H], bf16, tag="e")
                den = wk.tile([TT, H], f32, tag="den")
                rec = wk.tile([TT, H], f32, tag="rec")
                recb = wk.tile([TT, H], bf16, tag="recb")
                w = wk.tile([TT, H * H], bf16, tag="w")
                pv = wk.tile([TT, H * HD * H], bf16, tag="pv")
                nc.vector.tensor_tensor(
                    prod[:].rearrange("p (g g2 d) -> p g g2 d", g=H, g2=H),
                    q_ap.rearrange("p (g d) -> p g d", g=H).unsqueeze(2).broadcast_to([TT, H, H, HD]),
                    k_sb[:, koff:koff + C].rearrange("p (g2 d) -> p g2 d", g2=H).unsqueeze(1).broadcast_to([TT, H, H, HD]),
                    op=ALU.mult,
                )
                nc.vector.reduce_sum(
                    s[:].rearrange("p (g g2) -> p g g2", g=H),
                    prod[:].rearrange("p (g g2 d) -> p g g2 d", g=H, g2=H),
                    axis=AX.X,
                )
                nc.scalar.activation(e[:], s[:], AF.Exp)
                nc.vector.reduce_sum(den[:], e[:].rearrange("p (g g2) -> p g g2", g=H), axis=AX.X)
                nc.vector.reciprocal(rec[:], den[:])
                nc.vector.tensor_copy(recb[:], rec[:])
                nc.vector.tensor_tensor(
                    w[:].rearrange("p (g g2) -> p g g2", g=H),
                    e[:].rearrange("p (g g2) -> p g g2", g=H),
                    recb[:].unsqueeze(2).broadcast_to([TT, H, H]),
                    op=ALU.mult,
                )
                nc.vector.tensor_tensor(
                    pv[:].rearrange("p (g d g2) -> p g d g2", g=H, d=HD),
                    w[:].rearrange("p (g g2) -> p g g2", g=H).unsqueeze(2).broadcast_to([TT, H, HD, H]),
                    k_sb[:, voff:voff + C].rearrange("p (d g2) -> p d g2", d=HD).unsqueeze(1).broadcast_to([TT, H, HD, H]),
                    op=ALU.mult,
                )
                nc.vector.reduce_sum(
                    attn_out[:].rearrange("p (d g) -> p d g", d=HD).transpose([0, 2, 1]),
                    pv[:].rearrange("p (g d g2) -> p g d g2", g=H, d=HD),
                    axis=AX.X,
                )

            for b in range(BLOC):
                # reset per-batch accumulators
                nc.gpsimd.memset(mx1a[:], -1e30)
                nc.gpsimd.memset(mx2a[:], -1e30)
                nc.gpsimd.memset(mxca[:], -1e30)

                # ---------------- phase A: tiles ----------------
                for i in range(NT):
                    x1 = xin.tile([TT, C], f32, tag="x1")
                    x2 = xin.tile([TT, C], f32, tag="x2")
                    nc.sync.dma_start(x1[:], x_d[b, i * TT:(i + 1) * TT, :])
                    nc.sync.dma_start(x2[:], x_d[b, M + i * TT:M + (i + 1) * TT, :])

                    pstr = ps.tile([128, 2 * C], f32, tag="tr")
                    nc.tensor.transpose(pstr[:, 0:C], x1[:], identf[:])
                    nc.tensor.transpose(pstr[:, C:2 * C], x2[:], identf[:])
                    x1t = wk.tile([C, TT], bf16, tag="x1t")
                    x2t = wk.tile([C, TT], bf16, tag="x2t")
                    nc.scalar.copy(x1t[:], pstr[:, 0:C])
                    nc.vector.tensor_copy(x2t[:], pstr[:, C:2 * C])

                    psq = ps.tile([TT, 3 * C], f32, tag="qkv")
                    nc.tensor.matmul(psq[:], x1t[:], wqkv[:], start=True, stop=True)
                    psk = ps.tile([TT, 2 * C], f32, tag="kv2")
                    nc.tensor.matmul(psk[:], x2t[:], wqkv[:, C:3 * C], start=True, stop=True)

                    qkv1 = wk.tile([TT, 3 * C], bf16, tag="qkv1")
                    kv2 = wk.tile([TT, 2 * C], bf16, tag="kv2s")
                    nc.scalar.copy(qkv1[:], psq[:])
                    nc.vector.tensor_copy(kv2[:], psk[:])

                    attn1 = at.tile([TT, C], f32, tag="attn1")
                    attn2 = at.tile([TT, C], f32, tag="attn2")
                    attention((qkv1[:, 0:C], qkv1), C, 2 * C, attn1)
                    attention((qkv1[:, 0:C], kv2), 0, C, attn2)

                    psa = ps.tile([128, 2 * C], f32, tag="trA")
                    nc.tensor.transpose(psa[:, 0:C], attn1[:], identf[:])
                    nc.tensor.transpose(psa[:, C:2 * C], attn2[:], identf[:])

                    nc.scalar.activation(
                        a1cm[:, i * TT:(i + 1) * TT], psa[:, 0:C], AF.Copy,
                        accum_out=sum1c[:, i:i + 1],
                    )
                    nc.scalar.activation(
                        a2cm[:, i * TT:(i + 1) * TT], psa[:, C:2 * C], AF.Copy,
                        accum_out=sum2c[:, i:i + 1],
                    )
                    nc.vector.tensor_tensor(
                        acm[:, i * TT:(i + 1) * TT],
                        a1cm[:, i * TT:(i + 1) * TT],
                        a2cm[:, i * TT:(i + 1) * TT],
                        op=ALU.subtract,
                    )
                    nc.vector.tensor_tensor(mx1a[:], mx1a[:], a1cm[:, i * TT:(i + 1) * TT], op=ALU.max)
                    nc.vector.tensor_tensor(mx2a[:], mx2a[:], a2cm[:, i * TT:(i + 1) * TT], op=ALU.max)
                    nc.vector.tensor_tensor(mxca[:], mxca[:], acm[:, i * TT:(i + 1) * TT], op=ALU.max)

                # ---------------- phase B: stats, SE, A matrices ----------------
                s1 = wk.tile([C, 1], f32, tag="s1")
                s2 = wk.tile([C, 1], f32, tag="s2")
                scm = wk.tile([C, 1], f32, tag="scm")
                nc.vector.reduce_sum(s1[:], sum1c[:], axis=AX.X)
                nc.vector.reduce_sum(s2[:], sum2c[:], axis=AX.X)
                nc.vector.tensor_tensor(scm[:], s1[:], s2[:], op=ALU.subtract)
                mx1 = wk.tile([C, 1], f32, tag="mx1")
                mx2 = wk.tile([C, 1], f32, tag="mx2")
                mxc = wk.tile([C, 1], f32, tag="mxc")
                nc.vector.reduce_max(mx1[:], mx1a[:], axis=AX.X)
                nc.vector.reduce_max(mx2[:], mx2a[:], axis=AX.X)
                nc.vector.reduce_max(mxc[:], mxca[:], axis=AX.X)

                chs = []
                for t, (sv, mv) in enumerate([(scm, mxc), (s1, mx1), (s2, mx2)]):
                    stack = wk.tile([C, 2], bf16, tag="stack")
                    nc.scalar.activation(stack[:, 0:1], sv[:], AF.Copy, scale=1.0 / M)
                    nc.vector.tensor_copy(stack[:, 1:2], mv[:])
                    psfc = ps.tile([C // 4, 2], f32, tag="tr")
                    nc.tensor.matmul(psfc[:], caw1t[:], stack[:], start=True, stop=True)
                    relu2 = wk.tile([C // 4, 2], f32, tag="relu2")
                    nc.scalar.activation(relu2[:], psfc[:], AF.Relu)
                    u = wk.tile([C // 4, 1], bf16, tag="u")
                    nc.vector.tensor_tensor(u[:], relu2[:, 0:1], relu2[:, 1:2], op=ALU.add)
                    psch = ps.tile([C, 1], f32, tag="kv2")
                    nc.tensor.matmul(psch[:], caw2t[:], u[:], start=True, stop=True)
                    ch = wk.tile([C, 1], f32, tag=f"ch{t}")
                    nc.scalar.activation(ch[:], psch[:], AF.Sigmoid)
                    chs.append(ch)

                ch_row = []
                for t in range(3):
                    psr = ps.tile([1, C], f32, tag="trA")
                    nc.tensor.transpose(psr[:], chs[t][:], identf[:])
                    row = wk.tile([1, C], bf16, tag=f"row{t}")
                    nc.vector.tensor_copy(row[:], psr[:])
                    ch_row.append(row)

                pmats = []
                for t in (1, 2):
                    psat = ps.tile([C, C], f32, tag="qkv")
                    nc.tensor.matmul(psat[:], ch_row[t][:], ch_row[0][:], start=True, stop=True)
                    eat = wk.tile([C, C], bf16, tag="eat")
                    dena = wk.tile([C, 1], f32, tag="dena")
                    nc.scalar.activation(eat[:], psat[:], AF.Exp, accum_out=dena[:])
                    reca = wk.tile([C, 1], bf16, tag="reca")
                    recaf = wk.tile([C, 1], f32, tag="recaf")
                    nc.vector.reciprocal(recaf[:], dena[:])
                    nc.vector.tensor_copy(reca[:], recaf[:])
                    atsm = wk.tile([C, C], bf16, tag="atsm")
                    nc.vector.tensor_tensor(
                        atsm[:], eat[:],
                        reca[:].broadcast_to([C, C]),
                        op=ALU.mult,
                    )
                    psp = ps.tile([C, C], f32, tag="tr")
                    nc.tensor.matmul(psp[:], atsm[:], projp[:], start=True, stop=True)
                    pm = wk.tile([C, C], bf16, tag=f"pm{t}")
                    nc.scalar.copy(pm[:], psp[:])
                    pmats.append(pm)

                # ---------------- phase C: outputs ----------------
                for half in range(2):
                    resid = a1cm if half == 0 else a2cm
                    pmat = pmats[half]
                    for i in range(NT):
                        pso = ps.tile([TT, C], f32, tag="qkv")
                        nc.tensor.matmul(pso[:], ones_row[:], projb_row[:], start=True, stop=False)
                        nc.tensor.matmul(pso[:], acm[:, i * TT:(i + 1) * TT], pmat[:], start=False, stop=False)
                        nc.tensor.matmul(pso[:], resid[:, i * TT:(i + 1) * TT], projp[:], start=False, stop=True)
                        osb = op_pool.tile([TT, C], f32, tag="osb")
                        if i % 2 == 0:
                            nc.scalar.copy(osb[:], pso[:])
                        else:
                            nc.vector.tensor_copy(osb[:], pso[:])
                        nc.sync.dma_start(y_d[b, half * M + i * TT:half * M + (i + 1) * TT, :], osb[:])

    nc.compile()
    return nc


def _get_nc():
    if "nc" not in _CACHE:
        _CACHE["nc"] = _build()
    return _CACHE["nc"]


def kernel(**inputs):
    from concourse import bass_utils

    nc = _get_nc()
    x = np.ascontiguousarray(inputs["x"], dtype=np.float32)
    shared = {
        "qkv_w": np.ascontiguousarray(inputs["qkv_w"], dtype=np.float32),
        "ca_w1": np.ascontiguousarray(inputs["ca_w1"], dtype=np.float32),
        "ca_w2": np.ascontiguousarray(inputs["ca_w2"], dtype=np.float32),
        "proj_w": np.ascontiguousarray(inputs["proj_w"], dtype=np.float32),
        "proj_b": np.ascontiguousarray(inputs["proj_b"], dtype=np.float32),
    }
    in_maps = [
        {"x": x[c * BLOC:(c + 1) * BLOC], **shared} for c in range(NCORES)
    ]
    res = bass_utils.run_bass_kernel_spmd(nc, in_maps, core_ids=list(range(NCORES)))
    out = np.concatenate([res.results[c]["y"] for c in range(NCORES)], axis=0)
    return out
